# revision 2
# baseline (speedup 1.0000x reference)
"""Trainium2 Bass kernel for nn_Head (single-head causal attention).

Contract: kernel(**inputs) takes FULL inputs (x [8,2048,1024] f32,
Wk/Wq/Wv [64,1024] f32) and returns the FULL output [8,2048,64] f32.
Data-parallel over batch B=8 across the 8 NeuronCores (one batch row
per core); each core runs an identical single-core program.

Per-core dataflow (all bf16 on PE, f32 in PSUM):
  xt [C,T] streams in per 512-column chunk c; per chunk:
    kq projection: [Wk/32; Wq].T packed 128-wide -> kq_sb (k rows 0:64,
      q rows 64:128), then a PE partition-swap (two matmuls against
      identity column-halves) builds kq_dup (q rows 0:64, k rows 64:128)
      so score pairs can be row-tiled.
    v projection -> vt_sb, PE-transposed into vaug [128,16,65] with a
      ones column (row 64 of the PV output becomes the softmax
      denominator).
    attention: for s-tile pairs p=0..2c+1, TWO K=64 score matmuls run
      CONCURRENTLY in PE row groups 0:64 / 64:128 (tile_position row
      tiling), writing one [128,1024] PSUM pair tile; ONE Exp ACTIVATE
      per pair tile (scores pre-scaled by 1/sqrt(C) via Wk, no
      max-subtraction needed: |S| < 0.75); causal masking via an
      upper-triangular multiply on the two diagonal blocks (gpsimd);
      PV accumulates ot_ps[65,512] (one PSUM bank per chunk).
    epilogue: ot -> SBUF, PE transpose per 128-row tile, reciprocal of
      the denominator column, normalize, DMA out.
  Emission is software-pipelined: next-chunk projections and previous-
  chunk epilogues are interleaved as "filler" between the score pairs
  of the current chunk, so the in-order PE queue never stalls on the
  ACT exp chain; ~30 warmup matmuls run during the input DMA so the
  HAM clock gate reaches 2.4GHz before real compute.
"""

import sys

if "/opt/trn_rl_repo" not in sys.path:
    sys.path.insert(0, "/opt/trn_rl_repo")

import numpy as np

B = 8
T = 2048
C = 1024
H = 64
P = 128
CB = C // P
NCH = 4            # t-chunks of 512
NT = T // P        # 16 s-tiles
N_CORES = 8

_NC_CACHE = {}


def _build_nc():
    import concourse.bass as bass
    import concourse.mybir as mybir
    import concourse.tile as tile
    from concourse.bass import ts
    from concourse.masks import make_identity, make_upper_triangular

    fp32 = mybir.dt.float32
    bf16 = mybir.dt.bfloat16
    EXP = mybir.ActivationFunctionType.Exp

    nc = bass.Bass(target_bir_lowering=False, debug=False)
    xt_d = nc.declare_dram_parameter("xt", [P, CB, T], bf16, isOutput=False)
    wkq_d = nc.declare_dram_parameter("wkq", [P, CB, P], bf16, isOutput=False)
    wv_d = nc.declare_dram_parameter("wv", [P, CB, H], bf16, isOutput=False)
    out_d = nc.declare_dram_parameter("out", [T, H], fp32, isOutput=True)

    from contextlib import ExitStack

    with tile.TileContext(nc) as tc, ExitStack() as stk:
        pers = stk.enter_context(tc.tile_pool(name="pers", bufs=1))
        xt_sb = pers.tile([P, CB, T], bf16, tag="xt_sb", name="xt_sb")
        wkq_sb = pers.tile([P, CB, P], bf16, tag="wkq_sb", name="wkq_sb")
        wv_sb = pers.tile([P, CB, H], bf16, tag="wv_sb", name="wv_sb")
        # kq_sb rows 0:64 = kT (scaled), rows 64:128 = qT
        # kq_dup rows 0:64 = qT (copy),  rows 64:128 = kT (copy)
        kq_sb = pers.tile([P, T], bf16, tag="kq_sb", name="kq_sb")
        kq_dup = pers.tile([P, T], bf16, tag="kq_dup", name="kq_dup")
        vt_sb = pers.tile([H, T], fp32, tag="vt_sb", name="vt_sb")
        vaug_sb = pers.tile([P, NT, H + 1], bf16, tag="vaug_sb", name="vaug_sb")
        ot_sb = pers.tile([H + 1, T], fp32, tag="ot_sb", name="ot_sb")
        o_sb = pers.tile([P, NT, H], fp32, tag="o_sb", name="o_sb")
        identf = pers.tile([P, P], fp32, tag="identf", name="identf")
        identb = pers.tile([P, P], bf16, tag="identb", name="identb")
        warm_sb = pers.tile([P, P], bf16, tag="warm_sb", name="warm_sb")
        tri = pers.tile([P, P], bf16, tag="tri", name="tri")
        rec_sb = pers.tile([P, NT], fp32, tag="rec_sb", name="rec_sb")

        make_identity(nc, warm_sb[:])
        # ---- PE warmup while input DMAs stream: back-to-back matmuls so
        # HAM un-throttles to 2.4GHz before real compute ----
        with tc.tile_pool(name="warmp", bufs=1, space="PSUM") as warmp:
            wps = warmp.tile([P, P], fp32, tag="warm", name="warm")
            for w in range(30):
                nc.tensor.matmul(wps, warm_sb[:], warm_sb[:], start=True, stop=True)
        make_identity(nc, identf[:])
        make_identity(nc, identb[:])
        make_upper_triangular(nc, tri[:], val=1.0, diag=True)
        nc.any.memset(vaug_sb[:, :, H], 1.0)

        # ---- input DMAs (sync HWDGE ring, issued up front; chunk 0 in
        # four pieces so the first projection matmuls start ASAP) ----
        nc.sync.dma_start(wkq_sb[:], wkq_d[:])
        jc0 = slice(0, 512)
        for b4 in range(4):
            nc.sync.dma_start(
                xt_sb[:, 2 * b4 : 2 * b4 + 2, jc0],
                xt_d[:, 2 * b4 : 2 * b4 + 2, jc0])
        nc.sync.dma_start(wv_sb[:], wv_d[:])
        for c in range(1, NCH):
            jc = slice(512 * c, 512 * (c + 1))
            nc.sync.dma_start(xt_sb[:, :, jc], xt_d[:, :, jc])

        with (
            tc.tile_pool(name="projp", bufs=1, space="PSUM") as projp,
            tc.tile_pool(name="trp", bufs=2, space="PSUM") as trp,
            tc.tile_pool(name="stp", bufs=2, space="PSUM") as stp,
            tc.tile_pool(name="otp", bufs=1, space="PSUM") as otp,
            tc.tile_pool(name="ptp", bufs=4) as ptp,
        ):
            def proj_ops(c):
                """Projection for chunk c as a list of emission closures."""
                jc = slice(512 * c, 512 * (c + 1))
                ops = []
                kq_ps = projp.tile([P, 512], fp32, tag="kq", name=f"kq{c}")
                for cb in range(CB):
                    ops.append(lambda cb=cb, kq_ps=kq_ps, jc=jc: nc.tensor.matmul(
                        kq_ps, wkq_sb[:, cb, :], xt_sb[:, cb, jc],
                        start=(cb == 0), stop=(cb == CB - 1)))
                def kq_cast(kq_ps=kq_ps, jc=jc):
                    nc.vector.tensor_copy(kq_sb[:, jc], kq_ps)
                ops.append(kq_cast)
                dup_ps = projp.tile([P, 512], fp32, tag="kq", name=f"dup{c}")
                def kq_swap_q(dup_ps=dup_ps, jc=jc):
                    nc.tensor.matmul(
                        dup_ps[0:H, :], identb[:, H:P], kq_sb[:, jc],
                        start=True, stop=True)
                ops.append(kq_swap_q)
                def kq_swap_k(dup_ps=dup_ps, jc=jc):
                    nc.tensor.matmul(
                        dup_ps[H:P, :], identb[:, 0:H], kq_sb[:, jc],
                        start=True, stop=True)
                ops.append(kq_swap_k)
                ops.append(lambda dup_ps=dup_ps, jc=jc: nc.vector.tensor_copy(
                    kq_dup[:, jc], dup_ps))
                v_ps = projp.tile([P, 512], fp32, tag="kq", name=f"v{c}")
                for cb in range(CB):
                    ops.append(lambda cb=cb, v_ps=v_ps, jc=jc: nc.tensor.matmul(
                        v_ps[0:H, :], wv_sb[:, cb, :], xt_sb[:, cb, jc],
                        start=(cb == 0), stop=(cb == CB - 1)))
                ops.append(lambda v_ps=v_ps, jc=jc: nc.scalar.copy(
                    vt_sb[:, jc], v_ps[0:H, :]))
                for i in range(4 * c, 4 * c + 4):
                    def vtr_op(i=i):
                        vtr = trp.tile([P, H + 1], fp32, tag="tr", name=f"vtr{i}")
                        nc.tensor.transpose(
                            vtr[:, 0:H], vt_sb[:, ts(i, P)], identf[0:H, 0:H])
                        nc.vector.tensor_copy(vaug_sb[:, i, 0:H], vtr[:, 0:H])
                    ops.append(vtr_op)
                return ops

            def epi_ops(c):
                """Epilogue for chunk c (after its ot_sb copy) as closures."""
                ops = []
                for i in range(4 * c, 4 * c + 4):
                    def otr_op(i=i):
                        otr = trp.tile([P, H + 1], fp32, tag="tr", name=f"otr{i}")
                        nc.tensor.transpose(
                            otr, ot_sb[:, ts(i, P)], identf[0 : H + 1, 0 : H + 1])
                        nc.vector.reciprocal(
                            rec_sb[:, i : i + 1], otr[:, H : H + 1])
                        nc.any.tensor_scalar_mul(
                            o_sb[:, i, :], otr[:, 0:H], rec_sb[:, i : i + 1])
                    ops.append(otr_op)
                def dma_op(c=c):
                    nc.sync.dma_start(
                        out_d.rearrange("(i p) d -> p i d", p=P)[
                            :, 4 * c : 4 * c + 4, :],
                        o_sb[:, 4 * c : 4 * c + 4, :])
                ops.append(dma_op)
                return ops

            # chunk 0 projection runs un-interleaved at the head
            for op in proj_ops(0):
                op()

            for c in range(NCH):
                jc = slice(512 * c, 512 * (c + 1))
                pmax = 2 * c + 1
                # filler: previous chunk's epilogue + next chunk's projection
                filler = []
                if c >= 1:
                    filler += epi_ops(c - 1)
                if c + 1 < NCH:
                    filler += proj_ops(c + 1)

                ot_ps = otp.tile([H + 1, 512], fp32, tag="ot", name=f"ot{c}")

                def emit_pv(p, pt, c=c, ot_ps=ot_ps, pmax=pmax):
                    oA = max(0, 256 * p - 512 * c)
                    oB = max(0, 256 * p + 128 - 512 * c)
                    nc.tensor.matmul(
                        ot_ps[:, oA:512], vaug_sb[:, 2 * p, :], pt[:, oA:512],
                        start=(p == 0), stop=False)
                    nc.tensor.matmul(
                        ot_ps[:, oB:512], vaug_sb[:, 2 * p + 1, :],
                        pt[:, 512 + oB : 1024],
                        start=False, stop=(p == pmax))

                prev = None
                for p in range(pmax + 1):
                    c0 = p // 2
                    oA = max(0, 256 * p - 512 * c)
                    oB = max(0, 256 * p + 128 - 512 * c)
                    st = stp.tile([P, 1024], fp32, tag="st", name=f"st{c}_{p}")
                    pt = ptp.tile([P, 1024], bf16, tag="pt", name=f"pt{c}_{p}")
                    nc.tensor.matmul(
                        st[:, oA:512],
                        kq_dup[0:H, ts(2 * p, P)],
                        kq_sb[0:H, 512 * c + oA : 512 * (c + 1)],
                        start=True, stop=True)
                    nc.tensor.matmul(
                        st[:, 512 + oB : 1024],
                        kq_sb[H:P, ts(2 * p + 1, P)],
                        kq_dup[H:P, 512 * c + oB : 512 * (c + 1)],
                        start=True, stop=True)
                    nc.scalar.activation(pt[:, oA:1024], st[:, oA:1024], EXP)
                    if c == c0:
                        nc.gpsimd.tensor_tensor(
                            pt[:, oA : oA + P], pt[:, oA : oA + P], tri[:],
                            mybir.AluOpType.mult)
                        nc.gpsimd.tensor_tensor(
                            pt[:, 512 + oB : 512 + oB + P],
                            pt[:, 512 + oB : 512 + oB + P], tri[:],
                            mybir.AluOpType.mult)
                    # spread filler ops across the remaining ST slots; they
                    # sit between ST(p) and PV(p-1) in the PE queue so the
                    # exp PV(p-1) waits on has time to finish
                    nfill = -(-len(filler) // (pmax + 1 - p))
                    for op in filler[:nfill]:
                        op()
                    filler = filler[nfill:]
                    if prev is not None:
                        emit_pv(prev[0], prev[1])
                    prev = (p, pt)
                for op in filler:
                    op()
                emit_pv(prev[0], prev[1])
                prev = None
                nc.vector.tensor_copy(ot_sb[:, jc], ot_ps)

            for op in epi_ops(NCH - 1):
                op()

    return nc


def _split_multiwaits(nc):
    """Walrus codegen only supports one sync-wait command per instruction;
    hoist extra waits onto NoOps inserted just before (same engine queue,
    identical semantics since engines execute their queue in order)."""
    import concourse.mybir as mybir

    n = 0
    for fn in nc.m.functions:
        for block in fn.blocks:
            new_insts = []
            for inst in block.instructions:
                si = inst.sync_info
                if si is not None and si.on_wait and len(si.on_wait) > 1:
                    waits = list(si.on_wait)
                    for w in waits[:-1]:
                        n += 1
                        new_insts.append(
                            mybir.InstNoOp(
                                name=f"WH-{n}", engine=inst.engine, ins=[], outs=[],
                                sync_info=mybir.SyncInfo(on_wait=[w], on_update=[]),
                            )
                        )
                    si.on_wait = waits[-1:]
                new_insts.append(inst)
            block.instructions = new_insts
    return nc


def _get_nc():
    if "nc" not in _NC_CACHE:
        _NC_CACHE["nc"] = _split_multiwaits(_build_nc())
    return _NC_CACHE["nc"]


def _make_in_maps(x, Wk, Wq, Wv):
    import ml_dtypes

    bf16 = ml_dtypes.bfloat16
    scale = 1.0 / np.sqrt(np.float32(C))
    wkq = (
        np.concatenate([Wk * scale, Wq], axis=0).T.astype(bf16)
        .reshape(CB, P, P).transpose(1, 0, 2)
    )
    wkq = np.ascontiguousarray(wkq)
    wv = np.ascontiguousarray(
        Wv.T.astype(bf16).reshape(CB, P, H).transpose(1, 0, 2)
    )
    in_maps = []
    for b in range(B):
        xt = np.ascontiguousarray(
            x[b].T.astype(bf16).reshape(CB, P, T).transpose(1, 0, 2)
        )
        in_maps.append({"xt": xt, "wkq": wkq, "wv": wv})
    return in_maps


def run(x, Wk, Wq, Wv, trace=False):
    from concourse.bass_utils import run_bass_kernel_spmd

    nc = _get_nc()
    in_maps = _make_in_maps(x, Wk, Wq, Wv)
    res = run_bass_kernel_spmd(nc, in_maps, core_ids=list(range(N_CORES)), trace=trace)
    out = np.stack([np.asarray(res.results[b]["out"]) for b in range(B)], axis=0)
    return out.astype(np.float32), res


def kernel(x, Wk, Wq, Wv):
    out, _ = run(x, Wk, Wq, Wv, trace=False)
    return out


# revision 3
# speedup vs baseline: 1.0372x; 1.0372x over previous
"""Trainium2 Bass kernel for nn_Head (single-head causal attention).

Contract: kernel(**inputs) takes FULL inputs (x [8,2048,1024] f32,
Wk/Wq/Wv [64,1024] f32) and returns the FULL output [8,2048,64] f32.
Data-parallel over batch B=8 across the 8 NeuronCores (one batch row
per core); each core runs an identical single-core program.

Per-core dataflow (all bf16 on PE, f32 in PSUM):
  xt [C,T] streams in per 512-column chunk c; per chunk:
    kq projection: [Wk/32; Wq].T packed 128-wide -> kq_sb (k rows 0:64,
      q rows 64:128), then a PE partition-swap (two matmuls against
      identity column-halves) builds kq_dup (q rows 0:64, k rows 64:128)
      so score pairs can be row-tiled.
    v projection -> vt_sb, PE-transposed into vaug [128,16,65] with a
      ones column (row 64 of the PV output becomes the softmax
      denominator).
    attention: for s-tile pairs p=0..2c+1, TWO K=64 score matmuls run
      CONCURRENTLY in PE row groups 0:64 / 64:128 (tile_position row
      tiling), writing one [128,1024] PSUM pair tile; ONE Exp ACTIVATE
      per pair tile (scores pre-scaled by 1/sqrt(C) via Wk, no
      max-subtraction needed: |S| < 0.75); causal masking via an
      upper-triangular multiply on the two diagonal blocks (gpsimd);
      PV accumulates ot_ps[65,512] (one PSUM bank per chunk).
    epilogue: ot -> SBUF, PE transpose per 128-row tile, reciprocal of
      the denominator column, normalize, DMA out.
  Emission is software-pipelined: next-chunk projections and previous-
  chunk epilogues are interleaved as "filler" between the score pairs
  of the current chunk, so the in-order PE queue never stalls on the
  ACT exp chain; ~30 warmup matmuls run during the input DMA so the
  HAM clock gate reaches 2.4GHz before real compute.
"""

import sys

if "/opt/trn_rl_repo" not in sys.path:
    sys.path.insert(0, "/opt/trn_rl_repo")

import numpy as np

B = 8
T = 2048
C = 1024
H = 64
P = 128
CB = C // P
NCH = 4            # t-chunks of 512
NT = T // P        # 16 s-tiles
N_CORES = 8

_NC_CACHE = {}


def _build_nc():
    import concourse.bass as bass
    import concourse.mybir as mybir
    import concourse.tile as tile
    from concourse.bass import ts
    from concourse.masks import make_identity, make_upper_triangular

    fp32 = mybir.dt.float32
    bf16 = mybir.dt.bfloat16
    EXP = mybir.ActivationFunctionType.Exp

    nc = bass.Bass(target_bir_lowering=False, debug=False)
    xt_d = nc.declare_dram_parameter("xt", [P, CB, T], bf16, isOutput=False)
    wkq_d = nc.declare_dram_parameter("wkq", [P, CB, P], bf16, isOutput=False)
    wv_d = nc.declare_dram_parameter("wv", [P, CB, H], bf16, isOutput=False)
    out_d = nc.declare_dram_parameter("out", [T, H], fp32, isOutput=True)

    from contextlib import ExitStack

    with tile.TileContext(nc) as tc, ExitStack() as stk:
        pers = stk.enter_context(tc.tile_pool(name="pers", bufs=1))
        xt_sb = pers.tile([P, CB, T], bf16, tag="xt_sb", name="xt_sb")
        wkq_sb = pers.tile([P, CB, P], bf16, tag="wkq_sb", name="wkq_sb")
        wv_sb = pers.tile([P, CB, H], bf16, tag="wv_sb", name="wv_sb")
        # kq_sb rows 0:64 = kT (scaled), rows 64:128 = qT
        # kq_dup rows 0:64 = qT (copy),  rows 64:128 = kT (copy)
        kq_sb = pers.tile([P, T], bf16, tag="kq_sb", name="kq_sb")
        kq_dup = pers.tile([P, T], bf16, tag="kq_dup", name="kq_dup")
        vt_sb = pers.tile([H, T], fp32, tag="vt_sb", name="vt_sb")
        vaug_sb = pers.tile([P, NT, H + 1], bf16, tag="vaug_sb", name="vaug_sb")
        ot_sb = pers.tile([H + 1, T], fp32, tag="ot_sb", name="ot_sb")
        o_sb = pers.tile([P, NT, H], fp32, tag="o_sb", name="o_sb")
        identf = pers.tile([P, P], fp32, tag="identf", name="identf")
        identb = pers.tile([P, P], bf16, tag="identb", name="identb")
        warm_sb = pers.tile([P, P], bf16, tag="warm_sb", name="warm_sb")
        tri = pers.tile([P, P], bf16, tag="tri", name="tri")
        rec_sb = pers.tile([P, NT], fp32, tag="rec_sb", name="rec_sb")

        make_identity(nc, warm_sb[:])
        # ---- PE warmup while input DMAs stream: back-to-back matmuls so
        # HAM un-throttles to 2.4GHz before real compute ----
        with tc.tile_pool(name="warmp", bufs=1, space="PSUM") as warmp:
            wps = warmp.tile([P, P], fp32, tag="warm", name="warm")
            for w in range(30):
                nc.tensor.matmul(wps, warm_sb[:], warm_sb[:], start=True, stop=True)
        make_identity(nc, identf[:])
        make_identity(nc, identb[:])
        make_upper_triangular(nc, tri[:], val=1.0, diag=True)
        nc.any.memset(vaug_sb[:, :, H], 1.0)

        # ---- input DMAs (sync HWDGE ring, issued up front; chunk 0 in
        # four pieces so the first projection matmuls start ASAP) ----
        nc.sync.dma_start(wkq_sb[:], wkq_d[:])
        jc0 = slice(0, 512)
        for b4 in range(4):
            nc.sync.dma_start(
                xt_sb[:, 2 * b4 : 2 * b4 + 2, jc0],
                xt_d[:, 2 * b4 : 2 * b4 + 2, jc0])
        nc.sync.dma_start(wv_sb[:], wv_d[:])
        for c in range(1, NCH):
            jc = slice(512 * c, 512 * (c + 1))
            nc.sync.dma_start(xt_sb[:, :, jc], xt_d[:, :, jc])

        with (
            tc.tile_pool(name="projp", bufs=1, space="PSUM") as projp,
            tc.tile_pool(name="trp", bufs=2, space="PSUM") as trp,
            tc.tile_pool(name="stp", bufs=2, space="PSUM") as stp,
            tc.tile_pool(name="otp", bufs=1, space="PSUM") as otp,
            tc.tile_pool(name="ptp", bufs=6) as ptp,
        ):
            def proj_ops(c):
                """Projection for chunk c as a list of emission closures."""
                jc = slice(512 * c, 512 * (c + 1))
                ops = []
                kq_ps = projp.tile([P, 512], fp32, tag="kq", name=f"kq{c}")
                for cb in range(CB):
                    ops.append(lambda cb=cb, kq_ps=kq_ps, jc=jc: nc.tensor.matmul(
                        kq_ps, wkq_sb[:, cb, :], xt_sb[:, cb, jc],
                        start=(cb == 0), stop=(cb == CB - 1)))
                def kq_cast(kq_ps=kq_ps, jc=jc):
                    nc.vector.tensor_copy(kq_sb[:, jc], kq_ps)
                ops.append(kq_cast)
                dup_ps = projp.tile([P, 512], fp32, tag="kq", name=f"dup{c}")
                def kq_swap_q(dup_ps=dup_ps, jc=jc):
                    nc.tensor.matmul(
                        dup_ps[0:H, :], identb[:, H:P], kq_sb[:, jc],
                        start=True, stop=True)
                ops.append(kq_swap_q)
                def kq_swap_k(dup_ps=dup_ps, jc=jc):
                    nc.tensor.matmul(
                        dup_ps[H:P, :], identb[:, 0:H], kq_sb[:, jc],
                        start=True, stop=True)
                ops.append(kq_swap_k)
                ops.append(lambda dup_ps=dup_ps, jc=jc: nc.vector.tensor_copy(
                    kq_dup[:, jc], dup_ps))
                v_ps = projp.tile([P, 512], fp32, tag="kq", name=f"v{c}")
                for cb in range(CB):
                    ops.append(lambda cb=cb, v_ps=v_ps, jc=jc: nc.tensor.matmul(
                        v_ps[0:H, :], wv_sb[:, cb, :], xt_sb[:, cb, jc],
                        start=(cb == 0), stop=(cb == CB - 1)))
                ops.append(lambda v_ps=v_ps, jc=jc: nc.scalar.copy(
                    vt_sb[:, jc], v_ps[0:H, :]))
                for i in range(4 * c, 4 * c + 4):
                    def vtr_op(i=i):
                        vtr = trp.tile([P, H + 1], fp32, tag="tr", name=f"vtr{i}")
                        nc.tensor.transpose(
                            vtr[:, 0:H], vt_sb[:, ts(i, P)], identf[0:H, 0:H])
                        nc.vector.tensor_copy(vaug_sb[:, i, 0:H], vtr[:, 0:H])
                    ops.append(vtr_op)
                return ops

            def epi_ops(c):
                """Epilogue for chunk c (after its ot_sb copy) as closures."""
                ops = []
                for i in range(4 * c, 4 * c + 4):
                    def otr_op(i=i):
                        otr = trp.tile([P, H + 1], fp32, tag="tr", name=f"otr{i}")
                        nc.tensor.transpose(
                            otr, ot_sb[:, ts(i, P)], identf[0 : H + 1, 0 : H + 1])
                        nc.vector.reciprocal(
                            rec_sb[:, i : i + 1], otr[:, H : H + 1])
                        nc.any.tensor_scalar_mul(
                            o_sb[:, i, :], otr[:, 0:H], rec_sb[:, i : i + 1])
                    ops.append(otr_op)
                def dma_op(c=c):
                    nc.sync.dma_start(
                        out_d.rearrange("(i p) d -> p i d", p=P)[
                            :, 4 * c : 4 * c + 4, :],
                        o_sb[:, 4 * c : 4 * c + 4, :])
                ops.append(dma_op)
                return ops

            # chunk 0 projection runs un-interleaved at the head
            for op in proj_ops(0):
                op()

            for c in range(NCH):
                jc = slice(512 * c, 512 * (c + 1))
                pmax = 2 * c + 1
                # filler: previous chunk's epilogue + next chunk's projection
                filler = []
                if c >= 1:
                    filler += epi_ops(c - 1)
                if c + 1 < NCH:
                    filler += proj_ops(c + 1)

                ot_ps = otp.tile([H + 1, 512], fp32, tag="ot", name=f"ot{c}")

                def emit_pv(p, pt, c=c, ot_ps=ot_ps, pmax=pmax):
                    oA = max(0, 256 * p - 512 * c)
                    oB = max(0, 256 * p + 128 - 512 * c)
                    nc.tensor.matmul(
                        ot_ps[:, oA:512], vaug_sb[:, 2 * p, :], pt[:, oA:512],
                        start=(p == 0), stop=False)
                    nc.tensor.matmul(
                        ot_ps[:, oB:512], vaug_sb[:, 2 * p + 1, :],
                        pt[:, 512 + oB : 1024],
                        start=False, stop=(p == pmax))

                prev = None
                for p in range(pmax + 1):
                    c0 = p // 2
                    oA = max(0, 256 * p - 512 * c)
                    oB = max(0, 256 * p + 128 - 512 * c)
                    st = stp.tile([P, 1024], fp32, tag="st", name=f"st{c}_{p}")
                    pt = ptp.tile([P, 1024], bf16, tag="pt", name=f"pt{c}_{p}")
                    nc.tensor.matmul(
                        st[:, oA:512],
                        kq_dup[0:H, ts(2 * p, P)],
                        kq_sb[0:H, 512 * c + oA : 512 * (c + 1)],
                        start=True, stop=True)
                    nc.tensor.matmul(
                        st[:, 512 + oB : 1024],
                        kq_sb[H:P, ts(2 * p + 1, P)],
                        kq_dup[H:P, 512 * c + oB : 512 * (c + 1)],
                        start=True, stop=True)
                    nc.scalar.activation(pt[:, oA:1024], st[:, oA:1024], EXP)
                    if c == c0:
                        nc.gpsimd.tensor_tensor(
                            pt[:, oA : oA + P], pt[:, oA : oA + P], tri[:],
                            mybir.AluOpType.mult)
                        nc.gpsimd.tensor_tensor(
                            pt[:, 512 + oB : 512 + oB + P],
                            pt[:, 512 + oB : 512 + oB + P], tri[:],
                            mybir.AluOpType.mult)
                    # spread filler ops across the remaining ST slots; they
                    # sit between ST(p) and PV(p-1) in the PE queue so the
                    # exp PV(p-1) waits on has time to finish
                    nfill = -(-len(filler) // (pmax + 1 - p))
                    for op in filler[:nfill]:
                        op()
                    filler = filler[nfill:]
                    if prev is not None:
                        emit_pv(prev[0], prev[1])
                    prev = (p, pt)
                for op in filler:
                    op()
                emit_pv(prev[0], prev[1])
                prev = None
                nc.vector.tensor_copy(ot_sb[:, jc], ot_ps)

            for op in epi_ops(NCH - 1):
                op()

    return nc


def _split_multiwaits(nc):
    """Walrus codegen only supports one sync-wait command per instruction;
    hoist extra waits onto NoOps inserted just before (same engine queue,
    identical semantics since engines execute their queue in order)."""
    import concourse.mybir as mybir

    n = 0
    for fn in nc.m.functions:
        for block in fn.blocks:
            new_insts = []
            for inst in block.instructions:
                si = inst.sync_info
                if si is not None and si.on_wait and len(si.on_wait) > 1:
                    waits = list(si.on_wait)
                    for w in waits[:-1]:
                        n += 1
                        new_insts.append(
                            mybir.InstNoOp(
                                name=f"WH-{n}", engine=inst.engine, ins=[], outs=[],
                                sync_info=mybir.SyncInfo(on_wait=[w], on_update=[]),
                            )
                        )
                    si.on_wait = waits[-1:]
                new_insts.append(inst)
            block.instructions = new_insts
    return nc


def _get_nc():
    if "nc" not in _NC_CACHE:
        _NC_CACHE["nc"] = _split_multiwaits(_build_nc())
    return _NC_CACHE["nc"]


def _make_in_maps(x, Wk, Wq, Wv):
    import ml_dtypes

    bf16 = ml_dtypes.bfloat16
    scale = 1.0 / np.sqrt(np.float32(C))
    wkq = (
        np.concatenate([Wk * scale, Wq], axis=0).T.astype(bf16)
        .reshape(CB, P, P).transpose(1, 0, 2)
    )
    wkq = np.ascontiguousarray(wkq)
    wv = np.ascontiguousarray(
        Wv.T.astype(bf16).reshape(CB, P, H).transpose(1, 0, 2)
    )
    in_maps = []
    for b in range(B):
        xt = np.ascontiguousarray(
            x[b].T.astype(bf16).reshape(CB, P, T).transpose(1, 0, 2)
        )
        in_maps.append({"xt": xt, "wkq": wkq, "wv": wv})
    return in_maps


def run(x, Wk, Wq, Wv, trace=False):
    from concourse.bass_utils import run_bass_kernel_spmd

    nc = _get_nc()
    in_maps = _make_in_maps(x, Wk, Wq, Wv)
    res = run_bass_kernel_spmd(nc, in_maps, core_ids=list(range(N_CORES)), trace=trace)
    out = np.stack([np.asarray(res.results[b]["out"]) for b in range(B)], axis=0)
    return out.astype(np.float32), res


def kernel(x, Wk, Wq, Wv):
    out, _ = run(x, Wk, Wq, Wv, trace=False)
    return out


# revision 4
# speedup vs baseline: 1.0393x; 1.0020x over previous
"""Trainium2 Bass kernel for nn_Head (single-head causal attention).

Contract: kernel(**inputs) takes FULL inputs (x [8,2048,1024] f32,
Wk/Wq/Wv [64,1024] f32) and returns the FULL output [8,2048,64] f32.
Data-parallel over batch B=8 across the 8 NeuronCores (one batch row
per core); each core runs an identical single-core program.

Per-core dataflow (all bf16 on PE, f32 in PSUM):
  xt [C,T] streams in per 512-column chunk c; per chunk:
    kq projection: [Wk/32; Wq].T packed 128-wide -> kq_sb (k rows 0:64,
      q rows 64:128), then a PE partition-swap (two matmuls against
      identity column-halves) builds kq_dup (q rows 0:64, k rows 64:128)
      so score pairs can be row-tiled.
    v projection -> vt_sb, PE-transposed into vaug [128,16,65] with a
      ones column (row 64 of the PV output becomes the softmax
      denominator).
    attention: for s-tile pairs p=0..2c+1, TWO K=64 score matmuls run
      CONCURRENTLY in PE row groups 0:64 / 64:128 (tile_position row
      tiling), writing one [128,1024] PSUM pair tile; ONE Exp ACTIVATE
      per pair tile (scores pre-scaled by 1/sqrt(C) via Wk, no
      max-subtraction needed: |S| < 0.75); causal masking via an
      upper-triangular multiply on the two diagonal blocks (gpsimd);
      PV accumulates ot_ps[65,512] (one PSUM bank per chunk).
    epilogue: ot -> SBUF, PE transpose per 128-row tile, reciprocal of
      the denominator column, normalize, DMA out.
  Emission is software-pipelined: next-chunk projections and previous-
  chunk epilogues are interleaved as "filler" between the score pairs
  of the current chunk, so the in-order PE queue never stalls on the
  ACT exp chain; ~30 warmup matmuls run during the input DMA so the
  HAM clock gate reaches 2.4GHz before real compute.
"""

import sys

if "/opt/trn_rl_repo" not in sys.path:
    sys.path.insert(0, "/opt/trn_rl_repo")

import numpy as np

B = 8
T = 2048
C = 1024
H = 64
P = 128
CB = C // P
NCH = 4            # t-chunks of 512
NT = T // P        # 16 s-tiles
N_CORES = 8

_NC_CACHE = {}


def _build_nc():
    import concourse.bass as bass
    import concourse.mybir as mybir
    import concourse.tile as tile
    from concourse.bass import ts
    from concourse.masks import make_identity, make_upper_triangular

    fp32 = mybir.dt.float32
    bf16 = mybir.dt.bfloat16
    EXP = mybir.ActivationFunctionType.Exp

    nc = bass.Bass(target_bir_lowering=False, debug=False)
    xt_d = nc.declare_dram_parameter("xt", [P, CB, T], bf16, isOutput=False)
    wkq_d = nc.declare_dram_parameter("wkq", [P, CB, P], bf16, isOutput=False)
    wv_d = nc.declare_dram_parameter("wv", [P, CB, H], bf16, isOutput=False)
    out_d = nc.declare_dram_parameter("out", [T, H], fp32, isOutput=True)

    from contextlib import ExitStack

    with tile.TileContext(nc) as tc, ExitStack() as stk:
        pers = stk.enter_context(tc.tile_pool(name="pers", bufs=1))
        xt_sb = pers.tile([P, CB, T], bf16, tag="xt_sb", name="xt_sb")
        wkq_sb = pers.tile([P, CB, P], bf16, tag="wkq_sb", name="wkq_sb")
        wv_sb = pers.tile([P, CB, H], bf16, tag="wv_sb", name="wv_sb")
        # kq_sb rows 0:64 = kT (scaled), rows 64:128 = qT
        # kq_dup rows 0:64 = qT (copy),  rows 64:128 = kT (copy)
        kq_sb = pers.tile([P, T], bf16, tag="kq_sb", name="kq_sb")
        kq_dup = pers.tile([P, T], bf16, tag="kq_dup", name="kq_dup")
        vt_sb = pers.tile([H, T], fp32, tag="vt_sb", name="vt_sb")
        vaug_sb = pers.tile([P, NT, H + 1], bf16, tag="vaug_sb", name="vaug_sb")
        ot_sb = pers.tile([H + 1, T], fp32, tag="ot_sb", name="ot_sb")
        o_sb = pers.tile([P, NT, H], fp32, tag="o_sb", name="o_sb")
        identf = pers.tile([P, P], fp32, tag="identf", name="identf")
        identb = pers.tile([P, P], bf16, tag="identb", name="identb")
        warm_sb = pers.tile([P, P], bf16, tag="warm_sb", name="warm_sb")
        tri = pers.tile([P, P], bf16, tag="tri", name="tri")
        rec_sb = pers.tile([P, NT], fp32, tag="rec_sb", name="rec_sb")

        make_identity(nc, warm_sb[:])
        # ---- PE warmup while input DMAs stream: back-to-back matmuls so
        # HAM un-throttles to 2.4GHz before real compute ----
        with tc.tile_pool(name="warmp", bufs=1, space="PSUM") as warmp:
            wps = warmp.tile([P, P], fp32, tag="warm", name="warm")
            for w in range(36):
                nc.tensor.matmul(wps, warm_sb[:], warm_sb[:], start=True, stop=True)
        make_identity(nc, identf[:])
        make_identity(nc, identb[:])
        make_upper_triangular(nc, tri[:], val=1.0, diag=True)
        nc.any.memset(vaug_sb[:, :, H], 1.0)

        # ---- input DMAs (sync HWDGE ring, issued up front; chunk 0 in
        # four pieces so the first projection matmuls start ASAP) ----
        nc.sync.dma_start(wkq_sb[:], wkq_d[:])
        jc0 = slice(0, 512)
        for b4 in range(4):
            nc.sync.dma_start(
                xt_sb[:, 2 * b4 : 2 * b4 + 2, jc0],
                xt_d[:, 2 * b4 : 2 * b4 + 2, jc0])
        nc.sync.dma_start(wv_sb[:], wv_d[:])
        for c in range(1, NCH):
            jc = slice(512 * c, 512 * (c + 1))
            nc.sync.dma_start(xt_sb[:, :, jc], xt_d[:, :, jc])

        with (
            tc.tile_pool(name="projp", bufs=1, space="PSUM") as projp,
            tc.tile_pool(name="trp", bufs=2, space="PSUM") as trp,
            tc.tile_pool(name="stp", bufs=2, space="PSUM") as stp,
            tc.tile_pool(name="otp", bufs=1, space="PSUM") as otp,
            tc.tile_pool(name="ptp", bufs=6) as ptp,
        ):
            def proj_ops(c):
                """Projection for chunk c as a list of emission closures."""
                jc = slice(512 * c, 512 * (c + 1))
                ops = []
                kq_ps = projp.tile([P, 512], fp32, tag="kq", name=f"kq{c}")
                for cb in range(CB):
                    ops.append(lambda cb=cb, kq_ps=kq_ps, jc=jc: nc.tensor.matmul(
                        kq_ps, wkq_sb[:, cb, :], xt_sb[:, cb, jc],
                        start=(cb == 0), stop=(cb == CB - 1)))
                def kq_cast(kq_ps=kq_ps, jc=jc):
                    nc.vector.tensor_copy(kq_sb[:, jc], kq_ps)
                ops.append(kq_cast)
                dup_ps = projp.tile([P, 512], fp32, tag="kq", name=f"dup{c}")
                def kq_swap_q(dup_ps=dup_ps, jc=jc):
                    nc.tensor.matmul(
                        dup_ps[0:H, :], identb[:, H:P], kq_sb[:, jc],
                        start=True, stop=True)
                ops.append(kq_swap_q)
                def kq_swap_k(dup_ps=dup_ps, jc=jc):
                    nc.tensor.matmul(
                        dup_ps[H:P, :], identb[:, 0:H], kq_sb[:, jc],
                        start=True, stop=True)
                ops.append(kq_swap_k)
                ops.append(lambda dup_ps=dup_ps, jc=jc: nc.vector.tensor_copy(
                    kq_dup[:, jc], dup_ps))
                v_ps = projp.tile([P, 512], fp32, tag="kq", name=f"v{c}")
                for cb in range(CB):
                    ops.append(lambda cb=cb, v_ps=v_ps, jc=jc: nc.tensor.matmul(
                        v_ps[0:H, :], wv_sb[:, cb, :], xt_sb[:, cb, jc],
                        start=(cb == 0), stop=(cb == CB - 1)))
                ops.append(lambda v_ps=v_ps, jc=jc: nc.scalar.copy(
                    vt_sb[:, jc], v_ps[0:H, :]))
                for i in range(4 * c, 4 * c + 4):
                    def vtr_op(i=i):
                        vtr = trp.tile([P, H + 1], fp32, tag="tr", name=f"vtr{i}")
                        nc.tensor.transpose(
                            vtr[:, 0:H], vt_sb[:, ts(i, P)], identf[0:H, 0:H])
                        nc.vector.tensor_copy(vaug_sb[:, i, 0:H], vtr[:, 0:H])
                    ops.append(vtr_op)
                return ops

            def epi_ops(c):
                """Epilogue for chunk c (after its ot_sb copy) as closures."""
                ops = []
                for i in range(4 * c, 4 * c + 4):
                    def otr_op(i=i):
                        otr = trp.tile([P, H + 1], fp32, tag="tr", name=f"otr{i}")
                        nc.tensor.transpose(
                            otr, ot_sb[:, ts(i, P)], identf[0 : H + 1, 0 : H + 1])
                        nc.vector.reciprocal(
                            rec_sb[:, i : i + 1], otr[:, H : H + 1])
                        nc.any.tensor_scalar_mul(
                            o_sb[:, i, :], otr[:, 0:H], rec_sb[:, i : i + 1])
                    ops.append(otr_op)
                def dma_op(c=c):
                    nc.sync.dma_start(
                        out_d.rearrange("(i p) d -> p i d", p=P)[
                            :, 4 * c : 4 * c + 4, :],
                        o_sb[:, 4 * c : 4 * c + 4, :])
                ops.append(dma_op)
                return ops

            # chunk 0 projection runs un-interleaved at the head
            for op in proj_ops(0):
                op()

            for c in range(NCH):
                jc = slice(512 * c, 512 * (c + 1))
                pmax = 2 * c + 1
                # filler: previous chunk's epilogue + next chunk's projection
                filler = []
                if c >= 1:
                    filler += epi_ops(c - 1)
                if c + 1 < NCH:
                    filler += proj_ops(c + 1)

                ot_ps = otp.tile([H + 1, 512], fp32, tag="ot", name=f"ot{c}")

                def emit_pv(p, pt, c=c, ot_ps=ot_ps, pmax=pmax):
                    oA = max(0, 256 * p - 512 * c)
                    oB = max(0, 256 * p + 128 - 512 * c)
                    nc.tensor.matmul(
                        ot_ps[:, oA:512], vaug_sb[:, 2 * p, :], pt[:, oA:512],
                        start=(p == 0), stop=False)
                    nc.tensor.matmul(
                        ot_ps[:, oB:512], vaug_sb[:, 2 * p + 1, :],
                        pt[:, 512 + oB : 1024],
                        start=False, stop=(p == pmax))

                prev = None
                for p in range(pmax + 1):
                    c0 = p // 2
                    oA = max(0, 256 * p - 512 * c)
                    oB = max(0, 256 * p + 128 - 512 * c)
                    st = stp.tile([P, 1024], fp32, tag="st", name=f"st{c}_{p}")
                    pt = ptp.tile([P, 1024], bf16, tag="pt", name=f"pt{c}_{p}")
                    nc.tensor.matmul(
                        st[:, oA:512],
                        kq_dup[0:H, ts(2 * p, P)],
                        kq_sb[0:H, 512 * c + oA : 512 * (c + 1)],
                        start=True, stop=True)
                    nc.tensor.matmul(
                        st[:, 512 + oB : 1024],
                        kq_sb[H:P, ts(2 * p + 1, P)],
                        kq_dup[H:P, 512 * c + oB : 512 * (c + 1)],
                        start=True, stop=True)
                    nc.scalar.activation(pt[:, oA:1024], st[:, oA:1024], EXP)
                    if c == c0:
                        nc.gpsimd.tensor_tensor(
                            pt[:, oA : oA + P], pt[:, oA : oA + P], tri[:],
                            mybir.AluOpType.mult)
                        nc.gpsimd.tensor_tensor(
                            pt[:, 512 + oB : 512 + oB + P],
                            pt[:, 512 + oB : 512 + oB + P], tri[:],
                            mybir.AluOpType.mult)
                    # spread filler ops across the remaining ST slots; they
                    # sit between ST(p) and PV(p-1) in the PE queue so the
                    # exp PV(p-1) waits on has time to finish
                    nfill = -(-len(filler) // (pmax + 1 - p))
                    for op in filler[:nfill]:
                        op()
                    filler = filler[nfill:]
                    if prev is not None:
                        emit_pv(prev[0], prev[1])
                    prev = (p, pt)
                for op in filler:
                    op()
                emit_pv(prev[0], prev[1])
                prev = None
                nc.vector.tensor_copy(ot_sb[:, jc], ot_ps)

            for op in epi_ops(NCH - 1):
                op()

    return nc


def _split_multiwaits(nc):
    """Walrus codegen only supports one sync-wait command per instruction;
    hoist extra waits onto NoOps inserted just before (same engine queue,
    identical semantics since engines execute their queue in order)."""
    import concourse.mybir as mybir

    n = 0
    for fn in nc.m.functions:
        for block in fn.blocks:
            new_insts = []
            for inst in block.instructions:
                si = inst.sync_info
                if si is not None and si.on_wait and len(si.on_wait) > 1:
                    waits = list(si.on_wait)
                    for w in waits[:-1]:
                        n += 1
                        new_insts.append(
                            mybir.InstNoOp(
                                name=f"WH-{n}", engine=inst.engine, ins=[], outs=[],
                                sync_info=mybir.SyncInfo(on_wait=[w], on_update=[]),
                            )
                        )
                    si.on_wait = waits[-1:]
                new_insts.append(inst)
            block.instructions = new_insts
    return nc


def _get_nc():
    if "nc" not in _NC_CACHE:
        _NC_CACHE["nc"] = _split_multiwaits(_build_nc())
    return _NC_CACHE["nc"]


def _make_in_maps(x, Wk, Wq, Wv):
    import ml_dtypes

    bf16 = ml_dtypes.bfloat16
    scale = 1.0 / np.sqrt(np.float32(C))
    wkq = (
        np.concatenate([Wk * scale, Wq], axis=0).T.astype(bf16)
        .reshape(CB, P, P).transpose(1, 0, 2)
    )
    wkq = np.ascontiguousarray(wkq)
    wv = np.ascontiguousarray(
        Wv.T.astype(bf16).reshape(CB, P, H).transpose(1, 0, 2)
    )
    in_maps = []
    for b in range(B):
        xt = np.ascontiguousarray(
            x[b].T.astype(bf16).reshape(CB, P, T).transpose(1, 0, 2)
        )
        in_maps.append({"xt": xt, "wkq": wkq, "wv": wv})
    return in_maps


def run(x, Wk, Wq, Wv, trace=False):
    from concourse.bass_utils import run_bass_kernel_spmd

    nc = _get_nc()
    in_maps = _make_in_maps(x, Wk, Wq, Wv)
    res = run_bass_kernel_spmd(nc, in_maps, core_ids=list(range(N_CORES)), trace=trace)
    out = np.stack([np.asarray(res.results[b]["out"]) for b in range(B)], axis=0)
    return out.astype(np.float32), res


def kernel(x, Wk, Wq, Wv):
    out, _ = run(x, Wk, Wq, Wv, trace=False)
    return out


# revision 5
# speedup vs baseline: 1.0473x; 1.0077x over previous
"""Trainium2 Bass kernel for nn_Head (single-head causal attention).

Contract: kernel(**inputs) takes FULL inputs (x [8,2048,1024] f32,
Wk/Wq/Wv [64,1024] f32) and returns the FULL output [8,2048,64] f32.
Data-parallel over batch B=8 across the 8 NeuronCores (one batch row
per core); each core runs an identical single-core program.

Per-core dataflow (all bf16 on PE, f32 in PSUM):
  xt [C,T] streams in per 512-column chunk c; per chunk:
    kq projection: [Wk/32; Wq].T packed 128-wide -> kq_sb (k rows 0:64,
      q rows 64:128), then a PE partition-swap (two matmuls against
      identity column-halves) builds kq_dup (q rows 0:64, k rows 64:128)
      so score pairs can be row-tiled.
    v projection -> vt_sb, PE-transposed into vaug [128,16,65] with a
      ones column (row 64 of the PV output becomes the softmax
      denominator).
    attention: for s-tile pairs p=0..2c+1, TWO K=64 score matmuls run
      CONCURRENTLY in PE row groups 0:64 / 64:128 (tile_position row
      tiling), writing one [128,1024] PSUM pair tile; ONE Exp ACTIVATE
      per pair tile (scores pre-scaled by 1/sqrt(C) via Wk, no
      max-subtraction needed: |S| < 0.75); causal masking via an
      upper-triangular multiply on the two diagonal blocks (gpsimd);
      PV accumulates ot_ps[65,512] (one PSUM bank per chunk).
    epilogue: ot -> SBUF, PE transpose per 128-row tile, reciprocal of
      the denominator column, normalize, DMA out.
  Emission is software-pipelined: next-chunk projections and previous-
  chunk epilogues are interleaved as "filler" between the score pairs
  of the current chunk, so the in-order PE queue never stalls on the
  ACT exp chain; ~30 warmup matmuls run during the input DMA so the
  HAM clock gate reaches 2.4GHz before real compute.
"""

import sys

if "/opt/trn_rl_repo" not in sys.path:
    sys.path.insert(0, "/opt/trn_rl_repo")

import numpy as np

B = 8
T = 2048
C = 1024
H = 64
P = 128
CB = C // P
NCH = 4            # t-chunks of 512
NT = T // P        # 16 s-tiles
N_CORES = 8

_NC_CACHE = {}


def _build_nc():
    import concourse.bass as bass
    import concourse.mybir as mybir
    import concourse.tile as tile
    from concourse.bass import ts
    from concourse.masks import make_identity, make_upper_triangular

    fp32 = mybir.dt.float32
    bf16 = mybir.dt.bfloat16
    EXP = mybir.ActivationFunctionType.Exp

    nc = bass.Bass(target_bir_lowering=False, debug=False)
    xt_d = nc.declare_dram_parameter("xt", [P, CB, T], bf16, isOutput=False)
    wkq_d = nc.declare_dram_parameter("wkq", [P, CB, P], bf16, isOutput=False)
    wv_d = nc.declare_dram_parameter("wv", [P, CB, H], bf16, isOutput=False)
    out_d = nc.declare_dram_parameter("out", [T, H], fp32, isOutput=True)

    from contextlib import ExitStack

    with tile.TileContext(nc) as tc, ExitStack() as stk:
        pers = stk.enter_context(tc.tile_pool(name="pers", bufs=1))
        xt_sb = pers.tile([P, CB, T], bf16, tag="xt_sb", name="xt_sb")
        wkq_sb = pers.tile([P, CB, P], bf16, tag="wkq_sb", name="wkq_sb")
        wv_sb = pers.tile([P, CB, H], bf16, tag="wv_sb", name="wv_sb")
        # kq_sb rows 0:64 = kT (scaled), rows 64:128 = qT
        # kq_dup rows 0:64 = qT (copy),  rows 64:128 = kT (copy)
        kq_sb = pers.tile([P, T], bf16, tag="kq_sb", name="kq_sb")
        kq_dup = pers.tile([P, T], bf16, tag="kq_dup", name="kq_dup")
        vt_sb = pers.tile([H, T], fp32, tag="vt_sb", name="vt_sb")
        vaug_sb = pers.tile([P, NT, H + 1], bf16, tag="vaug_sb", name="vaug_sb")
        ot_sb = pers.tile([H + 1, T], fp32, tag="ot_sb", name="ot_sb")
        o_sb = pers.tile([P, NT, H], fp32, tag="o_sb", name="o_sb")
        identf = pers.tile([P, P], fp32, tag="identf", name="identf")
        identb = pers.tile([P, P], bf16, tag="identb", name="identb")
        warm_sb = pers.tile([P, P], bf16, tag="warm_sb", name="warm_sb")
        tri = pers.tile([P, P], bf16, tag="tri", name="tri")
        rec_sb = pers.tile([P, NT], fp32, tag="rec_sb", name="rec_sb")

        make_identity(nc, warm_sb[:])
        # ---- PE warmup while input DMAs stream: back-to-back matmuls so
        # HAM un-throttles to 2.4GHz before real compute ----
        with tc.tile_pool(name="warmp", bufs=1, space="PSUM") as warmp:
            wps = warmp.tile([P, P], fp32, tag="warm", name="warm")
            for w in range(36):
                nc.tensor.matmul(wps, warm_sb[:], warm_sb[:], start=True, stop=True)
        make_identity(nc, identf[:])
        make_identity(nc, identb[:])
        make_upper_triangular(nc, tri[:], val=1.0, diag=True)
        nc.any.memset(vaug_sb[:, :, H], 1.0)

        # ---- input DMAs (sync HWDGE ring, issued up front; chunk 0 in
        # four pieces so the first projection matmuls start ASAP) ----
        nc.sync.dma_start(wkq_sb[:], wkq_d[:])
        jc0 = slice(0, 512)
        for b4 in range(4):
            nc.sync.dma_start(
                xt_sb[:, 2 * b4 : 2 * b4 + 2, jc0],
                xt_d[:, 2 * b4 : 2 * b4 + 2, jc0])
        nc.sync.dma_start(wv_sb[:], wv_d[:])
        for c in range(1, NCH):
            jc = slice(512 * c, 512 * (c + 1))
            nc.sync.dma_start(xt_sb[:, :, jc], xt_d[:, :, jc])

        with (
            tc.tile_pool(name="projp", bufs=1, space="PSUM") as projp,
            tc.tile_pool(name="trp", bufs=2, space="PSUM") as trp,
            tc.tile_pool(name="stp", bufs=2, space="PSUM") as stp,
            tc.tile_pool(name="otp", bufs=1, space="PSUM") as otp,
            tc.tile_pool(name="ptp", bufs=6) as ptp,
        ):
            def proj_ops(c):
                """Projection for chunk c as a list of emission closures."""
                jc = slice(512 * c, 512 * (c + 1))
                ops = []
                kq_ps = projp.tile([P, 512], fp32, tag="kq", name=f"kq{c}")
                for cb in range(CB):
                    ops.append(lambda cb=cb, kq_ps=kq_ps, jc=jc: nc.tensor.matmul(
                        kq_ps, wkq_sb[:, cb, :], xt_sb[:, cb, jc],
                        start=(cb == 0), stop=(cb == CB - 1)))
                def kq_cast(kq_ps=kq_ps, jc=jc):
                    nc.vector.tensor_copy(kq_sb[:, jc], kq_ps)
                ops.append(kq_cast)
                dup_ps = projp.tile([P, 512], fp32, tag="kq", name=f"dup{c}")
                def kq_swap_q(dup_ps=dup_ps, jc=jc):
                    nc.tensor.matmul(
                        dup_ps[0:H, :], identb[:, H:P], kq_sb[:, jc],
                        start=True, stop=True)
                ops.append(kq_swap_q)
                def kq_swap_k(dup_ps=dup_ps, jc=jc):
                    nc.tensor.matmul(
                        dup_ps[H:P, :], identb[:, 0:H], kq_sb[:, jc],
                        start=True, stop=True)
                ops.append(kq_swap_k)
                ops.append(lambda dup_ps=dup_ps, jc=jc: nc.vector.tensor_copy(
                    kq_dup[:, jc], dup_ps))
                v_ps = projp.tile([P, 512], fp32, tag="kq", name=f"v{c}")
                for cb in range(CB):
                    ops.append(lambda cb=cb, v_ps=v_ps, jc=jc: nc.tensor.matmul(
                        v_ps[0:H, :], wv_sb[:, cb, :], xt_sb[:, cb, jc],
                        start=(cb == 0), stop=(cb == CB - 1)))
                ops.append(lambda v_ps=v_ps, jc=jc: nc.scalar.copy(
                    vt_sb[:, jc], v_ps[0:H, :]))
                for i in range(4 * c, 4 * c + 4):
                    def vtr_op(i=i):
                        vtr = trp.tile([P, H + 1], fp32, tag="tr", name=f"vtr{i}")
                        nc.tensor.transpose(
                            vtr[:, 0:H], vt_sb[:, ts(i, P)], identf[0:H, 0:H])
                        nc.vector.tensor_copy(vaug_sb[:, i, 0:H], vtr[:, 0:H])
                    ops.append(vtr_op)
                return ops

            def epi_ops(c):
                """Epilogue for chunk c (after its ot_sb copy) as closures."""
                ops = []
                for i in range(4 * c, 4 * c + 4):
                    def otr_op(i=i):
                        otr = trp.tile([P, H + 1], fp32, tag="tr", name=f"otr{i}")
                        nc.tensor.transpose(
                            otr, ot_sb[:, ts(i, P)], identf[0 : H + 1, 0 : H + 1])
                        nc.vector.reciprocal(
                            rec_sb[:, i : i + 1], otr[:, H : H + 1])
                        nc.vector.tensor_scalar_mul(
                            o_sb[:, i, :], otr[:, 0:H], rec_sb[:, i : i + 1])
                    ops.append(otr_op)
                def dma_op(c=c):
                    nc.sync.dma_start(
                        out_d.rearrange("(i p) d -> p i d", p=P)[
                            :, 4 * c : 4 * c + 4, :],
                        o_sb[:, 4 * c : 4 * c + 4, :])
                ops.append(dma_op)
                return ops

            # chunk 0 projection runs un-interleaved at the head
            for op in proj_ops(0):
                op()

            for c in range(NCH):
                jc = slice(512 * c, 512 * (c + 1))
                pmax = 2 * c + 1
                # filler: previous chunk's epilogue + next chunk's projection
                filler = []
                if c >= 1:
                    filler += epi_ops(c - 1)
                if c + 1 < NCH:
                    filler += proj_ops(c + 1)

                ot_ps = otp.tile([H + 1, 512], fp32, tag="ot", name=f"ot{c}")

                def emit_pv(p, pt, c=c, ot_ps=ot_ps, pmax=pmax):
                    oA = max(0, 256 * p - 512 * c)
                    oB = max(0, 256 * p + 128 - 512 * c)
                    nc.tensor.matmul(
                        ot_ps[:, oA:512], vaug_sb[:, 2 * p, :], pt[:, oA:512],
                        start=(p == 0), stop=False)
                    nc.tensor.matmul(
                        ot_ps[:, oB:512], vaug_sb[:, 2 * p + 1, :],
                        pt[:, 512 + oB : 1024],
                        start=False, stop=(p == pmax))

                prev = None
                for p in range(pmax + 1):
                    c0 = p // 2
                    oA = max(0, 256 * p - 512 * c)
                    oB = max(0, 256 * p + 128 - 512 * c)
                    st = stp.tile([P, 1024], fp32, tag="st", name=f"st{c}_{p}")
                    pt = ptp.tile([P, 1024], bf16, tag="pt", name=f"pt{c}_{p}")
                    nc.tensor.matmul(
                        st[:, oA:512],
                        kq_dup[0:H, ts(2 * p, P)],
                        kq_sb[0:H, 512 * c + oA : 512 * (c + 1)],
                        start=True, stop=True)
                    nc.tensor.matmul(
                        st[:, 512 + oB : 1024],
                        kq_sb[H:P, ts(2 * p + 1, P)],
                        kq_dup[H:P, 512 * c + oB : 512 * (c + 1)],
                        start=True, stop=True)
                    nc.scalar.activation(pt[:, oA:1024], st[:, oA:1024], EXP)
                    if c == c0:
                        nc.gpsimd.tensor_tensor(
                            pt[:, oA : oA + P], pt[:, oA : oA + P], tri[:],
                            mybir.AluOpType.mult)
                        nc.gpsimd.tensor_tensor(
                            pt[:, 512 + oB : 512 + oB + P],
                            pt[:, 512 + oB : 512 + oB + P], tri[:],
                            mybir.AluOpType.mult)
                    # spread filler ops across the remaining ST slots; they
                    # sit between ST(p) and PV(p-1) in the PE queue so the
                    # exp PV(p-1) waits on has time to finish
                    nfill = -(-len(filler) // (pmax + 1 - p))
                    for op in filler[:nfill]:
                        op()
                    filler = filler[nfill:]
                    if prev is not None:
                        emit_pv(prev[0], prev[1])
                    prev = (p, pt)
                for op in filler:
                    op()
                emit_pv(prev[0], prev[1])
                prev = None
                nc.vector.tensor_copy(ot_sb[:, jc], ot_ps)

            for op in epi_ops(NCH - 1):
                op()

    return nc


def _split_multiwaits(nc):
    """Walrus codegen only supports one sync-wait command per instruction;
    hoist extra waits onto NoOps inserted just before (same engine queue,
    identical semantics since engines execute their queue in order)."""
    import concourse.mybir as mybir

    n = 0
    for fn in nc.m.functions:
        for block in fn.blocks:
            new_insts = []
            for inst in block.instructions:
                si = inst.sync_info
                if si is not None and si.on_wait and len(si.on_wait) > 1:
                    waits = list(si.on_wait)
                    for w in waits[:-1]:
                        n += 1
                        new_insts.append(
                            mybir.InstNoOp(
                                name=f"WH-{n}", engine=inst.engine, ins=[], outs=[],
                                sync_info=mybir.SyncInfo(on_wait=[w], on_update=[]),
                            )
                        )
                    si.on_wait = waits[-1:]
                new_insts.append(inst)
            block.instructions = new_insts
    return nc


def _get_nc():
    if "nc" not in _NC_CACHE:
        _NC_CACHE["nc"] = _split_multiwaits(_build_nc())
    return _NC_CACHE["nc"]


def _make_in_maps(x, Wk, Wq, Wv):
    import ml_dtypes

    bf16 = ml_dtypes.bfloat16
    scale = 1.0 / np.sqrt(np.float32(C))
    wkq = (
        np.concatenate([Wk * scale, Wq], axis=0).T.astype(bf16)
        .reshape(CB, P, P).transpose(1, 0, 2)
    )
    wkq = np.ascontiguousarray(wkq)
    wv = np.ascontiguousarray(
        Wv.T.astype(bf16).reshape(CB, P, H).transpose(1, 0, 2)
    )
    in_maps = []
    for b in range(B):
        xt = np.ascontiguousarray(
            x[b].T.astype(bf16).reshape(CB, P, T).transpose(1, 0, 2)
        )
        in_maps.append({"xt": xt, "wkq": wkq, "wv": wv})
    return in_maps


def run(x, Wk, Wq, Wv, trace=False):
    from concourse.bass_utils import run_bass_kernel_spmd

    nc = _get_nc()
    in_maps = _make_in_maps(x, Wk, Wq, Wv)
    res = run_bass_kernel_spmd(nc, in_maps, core_ids=list(range(N_CORES)), trace=trace)
    out = np.stack([np.asarray(res.results[b]["out"]) for b in range(B)], axis=0)
    return out.astype(np.float32), res


def kernel(x, Wk, Wq, Wv):
    out, _ = run(x, Wk, Wq, Wv, trace=False)
    return out
